# revision 13
# baseline (speedup 1.0000x reference)
"""Sliding-window MQA attention block on Trainium2 (single NeuronCore).

The full problem (batch 2 x 2048 tokens) is processed as 8 sequential
chunk-bodies of 512 query tokens on ONE core. Measured through this
container's axon-tunneled PJRT stack, per-execution dispatch cost scales
with the number of participating devices (~1.4 ms at 1 device vs ~6.4 ms
at 8) while the whole problem's device compute (~0.5 ms) fits inside a
single device's dispatch shadow -- so one core minimizes end-to-end
latency even though 8 cores are available. The chunk loop is
instruction-level parallel: the Tile scheduler overlaps chunk i+1's
DMA/projections with chunk i's attention/output.

Each chunk-body sees its 512 query tokens plus a 256-token K/V halo
(768 KV tokens, zero-padded in front for chunk 0 of each batch).
Shared weights are DMA'd to SBUF once; only the per-chunk activations
(x^T slice + kv-validity) stream per body.

Device algorithm per chunk, logits computed TRANSPOSED ([s, t]) so no
PE transposes of probs are needed:
  qT[1024, 512]  = WqT.T @ xqT            (per 128-row blocks; [hd, t])
  ktd[128, 768]  = K^T duplicated into both partition halves (MQA shared)
  vaug[128,6,65] = V with a validity column (gives softmax denominators
                   AND zeroes out the padded kv positions of chunk 0)
  per head-pair, per 128-query block tb (window = 3 kv blocks, diagonal):
    logitsT[s,t] blocks via matmul(lhsT=ktd[hd, s-blk], rhs=qT[hd, t-blk])
    probsT = exp(0.125 * logitsT)
    two triangular 128x128 masks (k3=0 lower, k3=2 upper) in one strided
    vector multiply; the middle diagonal needs no mask
    po[t, 65] += probsT_blk.T @ vaug_blk  (PE, accumulate 3 diagonals)
    attn[t, 64h:64h+64] = po[:, :64] * (1 / po[:, 64])
  attnT via PE transpose; final[512, 1024] = attnT.T @ WfT + bias
"""

import math
import os
import sys

import numpy as np

for _p in ("/opt/trn_rl_repo",):
    if _p not in sys.path and os.path.isdir(_p):
        sys.path.insert(0, _p)

import ml_dtypes

import concourse.bass as bass
import concourse.mybir as mybir
import concourse.tile as tile
from concourse import bacc
from concourse.bass_utils import run_bass_kernel_spmd
from concourse.masks import make_identity

WIDTH = 1024
H = 16
HD = 64
WIN = 256
T = 512          # query tokens per chunk
KV = 768         # kv tokens per chunk (256 halo + 512)
NKB = WIDTH // 128
NTB = T // 128
NSB = KV // 128
CHUNKS = 8       # 2 batches x 4 query chunks
F32 = mybir.dt.float32
DT = mybir.dt.bfloat16
NPDT = ml_dtypes.bfloat16

# packed input layout (bf16 columns): shared weight block, then one
# activation block per chunk
S_WK = 0                          # 8 x 64
S_WV = S_WK + NKB * HD            # 8 x 64
S_WQ = S_WV + NKB * HD            # 8 x 1024
S_TRI = S_WQ + NKB * WIDTH        # [128, 512]: lo|hi|lo|hi
S_BIAS = S_TRI + 512              # [128, 1024] replicated row
LEN_S1 = S_BIAS + WIDTH           # staging tile S1 (wk|wv|wq|tri|bias)
S_WF = LEN_S1                     # 8 x 1024
LEN_S = S_WF + NKB * WIDTH        # shared block total (18944)
CK_XKV = 0                        # 8 x 768 (within a chunk block)
CK_VALID = CK_XKV + NKB * KV      # [128, 8] (6 used)
LEN_CK = CK_VALID + 8             # per-chunk block total (6152)
PK_COLS = LEN_S + CHUNKS * LEN_CK



def build_kernel(reps=1):
    nc = bacc.Bacc(None, target_bir_lowering=False)

    pk_d = nc.dram_tensor("pk", [128, PK_COLS], DT, kind="ExternalInput")
    out_d = nc.dram_tensor("out", [CHUNKS * T, WIDTH], F32, kind="ExternalOutput")

    with tile.TileContext(nc) as tc:
        for rep in range(reps):
            _build_rep(nc, tc, rep, pk_d, out_d)
    return nc


def _emit_qproj_block(nc, shared, state, pool, mb):
    """One 128-row block of the Q projection: 8 matmuls + PSUM->SBUF copy."""
    pq = pool.tile([128, T], F32, tag="pq")
    for kb in range(NKB):
        nc.tensor.matmul(
            pq[:],
            lhsT=shared["wq"][kb][:, 128 * mb : 128 * (mb + 1)],
            rhs=state["xkv"][kb][:, WIN : WIN + T],
            start=(kb == 0),
            stop=(kb == NKB - 1),
        )
    nc.scalar.copy(state["qT"][mb][:], pq[:])


def _build_rep(nc, tc, rep, pk_d, out_d):
    with tc.tile_pool(name=f"shared{rep}", bufs=1) as sp:
        pkS1 = sp.tile([128, LEN_S1], DT, tag="pkS1")
        nc.sync.dma_start(pkS1[:], pk_d[:, 0:LEN_S1])
        pkS2 = sp.tile([128, LEN_S - LEN_S1], DT, tag="pkS2")
        nc.sync.dma_start(pkS2[:], pk_d[:, LEN_S1:LEN_S])

        shared = {
            "wk": [pkS1[:, S_WK + HD * i : S_WK + HD * (i + 1)] for i in range(NKB)],
            "wv": [pkS1[:, S_WV + HD * i : S_WV + HD * (i + 1)] for i in range(NKB)],
            "wq": [pkS1[:, S_WQ + WIDTH * i : S_WQ + WIDTH * (i + 1)] for i in range(NKB)],
            "tri": pkS1[:, S_TRI : S_TRI + 512],
            "bias": pkS1[:, S_BIAS : S_BIAS + WIDTH],
            "wf": [pkS2[:, WIDTH * i : WIDTH * (i + 1)] for i in range(NKB)],
        }
        ident = sp.tile([128, 128], DT, tag="ident")
        make_identity(nc, ident[:])
        bias_f = sp.tile([128, WIDTH], F32, tag="biasf")
        nc.scalar.copy(bias_f[:], shared["bias"])
        shared["bias_f"] = bias_f
        shared["ident"] = ident

        # rolling per-chunk in-flight state (activation slice + qT tiles):
        # a bufs=2 pool rotates two physical buffers, so state(c+1) can be
        # DMA'd and Q-projected while state(c) is still being consumed.
        with (
            tc.tile_pool(name=f"flight{rep}", bufs=2) as flight,
            tc.tile_pool(name=f"loc{rep}", bufs=2) as loc,
        ):
            states = [None] * CHUNKS

            def open_state(c):
                pkX = flight.tile([128, LEN_CK], DT, tag="pkX")
                nc.sync.dma_start(
                    pkX[:], pk_d[:, LEN_S + c * LEN_CK : LEN_S + (c + 1) * LEN_CK]
                )
                states[c] = {
                    "pkX": pkX,
                    "qT": [
                        flight.tile([128, T], DT, tag=f"qT{i}", name=f"qT{i}")
                        for i in range(NKB)
                    ],
                    "xkv": [pkX[:, KV * i : KV * (i + 1)] for i in range(NKB)],
                    "valid": pkX[:, CK_VALID : CK_VALID + NSB],
                }

            open_state(0)
            with tc.tile_pool(name=f"psq{rep}i", bufs=3, space="PSUM") as psq0:
                for mb in range(NKB):
                    _emit_qproj_block(nc, shared, states[0], psq0, mb)

            for c in range(CHUNKS):
                if c + 1 < CHUNKS:
                    open_state(c + 1)
                _build_body(
                    nc, tc, rep, out_d, shared, c, loc, states[c],
                    states[c + 1] if c + 1 < CHUNKS else None,
                )
                states[c] = None


def _build_body(nc, tc, rep, out_d, shared, c, loc, state, next_state):
    row0 = c * T
    wk, wv, wf = shared["wk"], shared["wv"], shared["wf"]
    tri, bias_f, ident = shared["tri"], shared["bias_f"], shared["ident"]
    xkv, valid, qT_t = state["xkv"], state["valid"], state["qT"]

    if True:
        ktd = loc.tile([128, KV], DT, tag="ktd")
        vaug = loc.tile([128, NSB, HD + 1], DT, tag="vaug")
        attn_t = [
            loc.tile([128, WIDTH], DT, tag=f"attn{i}", name=f"attn{i}")
            for i in range(NTB)
        ]
        attnT_t = [
            loc.tile([128, T], DT, tag=f"attnT{i}", name=f"attnT{i}")
            for i in range(NKB)
        ]

        # ---- K/V projections ----
        with (
            tc.tile_pool(name=f"psk{rep}c{c}", bufs=1, space="PSUM") as psk_pool,
            tc.tile_pool(name=f"psv{rep}c{c}", bufs=1, space="PSUM") as psv_pool,
        ):
            pk_ps = psk_pool.tile([128, KV], F32, tag="pk")
            for half in (0, 64):
                for seg0, segw in ((0, 512), (512, 256)):
                    for kb in range(NKB):
                        nc.tensor.matmul(
                            pk_ps[half : half + 64, seg0 : seg0 + segw],
                            lhsT=wk[kb],
                            rhs=xkv[kb][:, seg0 : seg0 + segw],
                            start=(kb == 0),
                            stop=(kb == NKB - 1),
                        )
            nc.vector.tensor_copy(ktd[:], pk_ps[:])

            pv = psv_pool.tile([128, NSB, HD], F32, tag="pv")
            for sb in range(NSB):
                for kb in range(NKB):
                    nc.tensor.matmul(
                        pv[:, sb, :],
                        lhsT=xkv[kb][:, 128 * sb : 128 * (sb + 1)],
                        rhs=wv[kb],
                        start=(kb == 0),
                        stop=(kb == NKB - 1),
                    )
            nc.scalar.copy(vaug[:, :, 0:HD], pv[:])
            nc.vector.tensor_copy(
                vaug[:, :, HD : HD + 1], valid.rearrange("p (s o) -> p s o", o=1)
            )

        # ---- attention, with next chunk's Q-projection software-pipelined
        # into the head-pair loop (one block per pair; PE fills ACT/DVE gaps)
        J2K3 = (0, 2, 1)
        with (
            tc.tile_pool(name=f"psl{rep}c{c}", bufs=2, space="PSUM") as psl_pool,
            tc.tile_pool(name=f"pso{rep}c{c}", bufs=2, space="PSUM") as pso_pool,
            tc.tile_pool(name=f"psqn{rep}c{c}", bufs=1, space="PSUM") as psqn_pool,
            tc.tile_pool(name=f"awork{rep}c{c}", bufs=3) as awork,
        ):
            for mb in range(NKB):  # head pair (2*mb, 2*mb+1)
                if next_state is not None:
                    _emit_qproj_block(nc, shared, next_state, psqn_pool, mb)
                qh = qT_t[mb]
                for tb in range(NTB):
                    pl = psl_pool.tile([128, 2, 4, 128], F32, tag="pl")
                    for half in (0, 1):
                        hb = 64 * half
                        for j in range(3):
                            sb = tb + J2K3[j]
                            nc.tensor.matmul(
                                pl[:, half, j, :],
                                lhsT=ktd[hb : hb + 64, 128 * sb : 128 * (sb + 1)],
                                rhs=qh[hb : hb + 64, 128 * tb : 128 * (tb + 1)],
                                start=True,
                                stop=True,
                            )
                    probsT = awork.tile([128, 2, 3, 128], DT, tag="probsT")
                    nc.scalar.activation(
                        out=probsT[:].rearrange("p h j t -> p h (j t)"),
                        in_=pl[:, :, 0:3, :].rearrange("p h j t -> p h (j t)"),
                        func=mybir.ActivationFunctionType.Exp,
                        scale=0.125,
                    )
                    for half in (0, 1):
                        nc.vector.tensor_mul(
                            probsT[:, half, 0:2, :],
                            probsT[:, half, 0:2, :],
                            tri[:, 0:256].rearrange("p (j t) -> p j t", j=2),
                        )
                    po = pso_pool.tile([128, 2, 128], F32, tag="po")
                    for half in (0, 1):
                        for j in range(3):
                            k3 = J2K3[j]
                            nc.tensor.matmul(
                                po[:, half, 0 : HD + 1],
                                lhsT=probsT[:, half, j, :],
                                rhs=vaug[:, tb + k3, :],
                                start=(j == 0),
                                stop=(j == 2),
                            )
                    recip = awork.tile([128, 2, 1], F32, tag="recip")
                    nc.vector.reciprocal(recip[:], po[:, :, HD : HD + 1])
                    for half in (0, 1):
                        h = 2 * mb + half
                        nc.vector.tensor_scalar_mul(
                            attn_t[tb][:, 64 * h : 64 * (h + 1)],
                            po[:, half, 0:HD],
                            recip[:, half, :],
                        )

        # attn -> attnT for the final projection
        with (
            tc.tile_pool(name=f"psat{rep}c{c}", bufs=2, space="PSUM") as psat_pool,
        ):
            for wb in range(NKB):
                pat = psat_pool.tile([128, NTB, 128], DT, tag="pat")
                for tb in range(NTB):
                    nc.tensor.transpose(
                        pat[:, tb, :],
                        attn_t[tb][:, 128 * wb : 128 * (wb + 1)],
                        ident[:],
                    )
                nc.vector.tensor_copy(attnT_t[wb][:], pat[:])

        # ---- final projection + bias ----
        with (
            tc.tile_pool(name=f"psf{rep}c{c}", bufs=4, space="PSUM") as psf_pool,
            tc.tile_pool(name=f"fin{rep}c{c}", bufs=3) as fin_pool,
        ):
            for tb in range(NTB):
                for nh in range(2):
                    pf = psf_pool.tile([128, 512], F32, tag="pf")
                    for wb in range(NKB):
                        nc.tensor.matmul(
                            pf[:],
                            lhsT=attnT_t[wb][:, 128 * tb : 128 * (tb + 1)],
                            rhs=wf[wb][:, 512 * nh : 512 * (nh + 1)],
                            start=(wb == 0),
                            stop=(wb == NKB - 1),
                        )
                    fo = fin_pool.tile([128, 512], F32, tag="fo")
                    nc.vector.tensor_add(
                        fo[:], pf[:], bias_f[:, 512 * nh : 512 * (nh + 1)]
                    )
                    nc.sync.dma_start(
                        out_d[
                            row0 + 128 * tb : row0 + 128 * (tb + 1),
                            512 * nh : 512 * (nh + 1),
                        ],
                        fo[:],
                    )


def prep_inputs(x, Wq, Wk, Wv, Wf, bf):
    """Pack everything into one [128, PK_COLS] bf16 array."""
    pk = np.zeros((128, PK_COLS), np.float32)
    for i in range(NKB):
        pk[:, S_WK + HD * i : S_WK + HD * (i + 1)] = Wk.T[128 * i : 128 * (i + 1)]
        pk[:, S_WV + HD * i : S_WV + HD * (i + 1)] = Wv.T[128 * i : 128 * (i + 1)]
        pk[:, S_WQ + WIDTH * i : S_WQ + WIDTH * (i + 1)] = Wq.T[128 * i : 128 * (i + 1)]
        pk[:, S_WF + WIDTH * i : S_WF + WIDTH * (i + 1)] = Wf.T[128 * i : 128 * (i + 1)]

    s = np.arange(128)[:, None]
    t = np.arange(128)[None, :]
    tri_lo = (s >= t).astype(np.float32)  # k3=0 block: keep s >= t
    tri_hi = (s <= t).astype(np.float32)  # k3=2 block: keep s <= t
    pk[:, S_TRI + 0 : S_TRI + 128] = tri_lo
    pk[:, S_TRI + 128 : S_TRI + 256] = tri_hi
    pk[:, S_TRI + 256 : S_TRI + 384] = tri_lo
    pk[:, S_TRI + 384 : S_TRI + 512] = tri_hi
    pk[:, S_BIAS : S_BIAS + WIDTH] = np.broadcast_to(bf.astype(np.float32), (128, WIDTH))

    for c in range(CHUNKS):
        bi, ch = divmod(c, 4)
        qs = T * ch
        ks = qs - WIN
        base = LEN_S + c * LEN_CK
        xkvT = np.zeros((WIDTH, KV), np.float32)
        lo = max(ks, 0)
        xkvT[:, lo - ks :] = x[bi, lo : qs + T, :].T
        for i in range(NKB):
            pk[:, base + CK_XKV + KV * i : base + CK_XKV + KV * (i + 1)] = xkvT[
                128 * i : 128 * (i + 1)
            ]
        kv_pos = ks + (np.arange(NSB)[None, :] * 128 + np.arange(128)[:, None])
        pk[:, base + CK_VALID : base + CK_VALID + NSB] = (kv_pos >= 0).astype(
            np.float32
        )
    return {"pk": pk.astype(NPDT)}


_RUN_KW = {}  # test.py can inject trace=True etc.
_LAST_RESULT = [None]


def kernel(x, segment_pos, Wq, Wk, Wv, Wf, bf):
    x = np.asarray(x, np.float32)
    Wq = np.asarray(Wq, np.float32)
    Wk = np.asarray(Wk, np.float32)
    Wv = np.asarray(Wv, np.float32)
    Wf = np.asarray(Wf, np.float32)
    bf = np.asarray(bf, np.float32)

    nc = build_kernel()
    nc.finalize()
    in_maps = [prep_inputs(x, Wq, Wk, Wv, Wf, bf)]
    res = run_bass_kernel_spmd(nc, in_maps, core_ids=[0], **_RUN_KW)
    _LAST_RESULT[0] = res

    b, t = x.shape[0], x.shape[1]
    return np.ascontiguousarray(
        res.results[0]["out"].reshape(b, t, WIDTH).astype(np.float32)
    )


# revision 14
# speedup vs baseline: 1.2697x; 1.2697x over previous
"""Sliding-window MQA attention block on Trainium2 (single NeuronCore).

The full problem (batch 2 x 2048 tokens) is processed as 8 sequential
chunk-bodies of 512 query tokens on ONE core. Measured through this
container's axon-tunneled PJRT stack, per-execution dispatch cost scales
with the number of participating devices (~1.4 ms at 1 device vs ~6.4 ms
at 8) while the whole problem's device compute (~0.5 ms) fits inside a
single device's dispatch shadow -- so one core minimizes end-to-end
latency even though 8 cores are available. The chunk loop is
instruction-level parallel: the Tile scheduler overlaps chunk i+1's
DMA/projections with chunk i's attention/output.

Each chunk-body sees its 512 query tokens plus a 256-token K/V halo
(768 KV tokens, zero-padded in front for chunk 0 of each batch).
Shared weights are DMA'd to SBUF once; only the per-chunk activations
(x^T slice + kv-validity) stream per body.

Device algorithm per chunk, logits computed TRANSPOSED ([s, t]) so no
PE transposes of probs are needed:
  qT[1024, 512]  = WqT.T @ xqT            (per 128-row blocks; [hd, t])
  ktd[128, 768]  = K^T duplicated into both partition halves (MQA shared)
  vaug[128,6,65] = V with a validity column (gives softmax denominators
                   AND zeroes out the padded kv positions of chunk 0)
  per head-pair, per 128-query block tb (window = 3 kv blocks, diagonal):
    logitsT[s,t] blocks via matmul(lhsT=ktd[hd, s-blk], rhs=qT[hd, t-blk])
    probsT = exp(0.125 * logitsT)
    two triangular 128x128 masks (k3=0 lower, k3=2 upper) in one strided
    vector multiply; the middle diagonal needs no mask
    po[t, 65] += probsT_blk.T @ vaug_blk  (PE, accumulate 3 diagonals)
    attn[t, 64h:64h+64] = po[:, :64] * (1 / po[:, 64])
  attnT via PE transpose; final[512, 1024] = attnT.T @ WfT + bias
"""

import math
import os
import sys

import numpy as np

for _p in ("/opt/trn_rl_repo",):
    if _p not in sys.path and os.path.isdir(_p):
        sys.path.insert(0, _p)

import ml_dtypes

import concourse.bass as bass
import concourse.mybir as mybir
import concourse.tile as tile
from concourse import bacc
from concourse.bass_utils import run_bass_kernel_spmd
from concourse.masks import make_identity

WIDTH = 1024
H = 16
HD = 64
WIN = 256
T = 512          # query tokens per chunk
KV = 768         # kv tokens per chunk (256 halo + 512)
NKB = WIDTH // 128
NTB = T // 128
NSB = KV // 128
CHUNKS = 8       # 2 batches x 4 query chunks
F32 = mybir.dt.float32
DT = mybir.dt.bfloat16
NPDT = ml_dtypes.bfloat16

# packed input layout (bf16 columns): shared weight block, then one
# activation block per chunk
S_WK = 0                          # 8 x 64
S_WV = S_WK + NKB * HD            # 8 x 64
S_WQ = S_WV + NKB * HD            # 8 x 1024
S_TRI = S_WQ + NKB * WIDTH        # [128, 512]: lo|hi|lo|hi
S_BIAS = S_TRI + 512              # [128, 1024] replicated row
LEN_S1 = S_BIAS + WIDTH           # staging tile S1 (wk|wv|wq|tri|bias)
S_WF = LEN_S1                     # 8 x 1024
LEN_S = S_WF + NKB * WIDTH        # shared block total (18944)
CK_XKV = 0                        # 8 x 768 (within a chunk block)
CK_VALID = CK_XKV + NKB * KV      # [128, 8] (6 used)
LEN_CK = CK_VALID + 8             # per-chunk block total (6152)
PK_COLS = LEN_S + CHUNKS * LEN_CK



def build_kernel(reps=1):
    nc = bacc.Bacc(None, target_bir_lowering=False)

    pk_d = nc.dram_tensor("pk", [128, PK_COLS], DT, kind="ExternalInput")
    out_d = nc.dram_tensor("out", [CHUNKS * T, WIDTH], F32, kind="ExternalOutput")

    with tile.TileContext(nc) as tc:
        for rep in range(reps):
            _build_rep(nc, tc, rep, pk_d, out_d)
    return nc


def _emit_qproj_block(nc, shared, state, pool, mb):
    """One 128-row block of the Q projection: 8 matmuls + PSUM->SBUF copy."""
    pq = pool.tile([128, T], F32, tag="pq")
    for kb in range(NKB):
        nc.tensor.matmul(
            pq[:],
            lhsT=shared["wq"][kb][:, 128 * mb : 128 * (mb + 1)],
            rhs=state["xkv"][kb][:, WIN : WIN + T],
            start=(kb == 0),
            stop=(kb == NKB - 1),
        )
    nc.scalar.copy(state["qT"][mb][:], pq[:])



def _emit_final_block(nc, shared, fin, pool, spool, mb):
    """One output block (tb = mb//2, nh = mb%2) of the PREVIOUS chunk's
    final projection: 8 matmuls + bias add + store."""
    tb, nh = divmod(mb, 2)
    pf = pool.tile([128, 512], F32, tag="pf")
    for wb in range(NKB):
        nc.tensor.matmul(
            pf[:],
            lhsT=fin["attnT"][wb][:, 128 * tb : 128 * (tb + 1)],
            rhs=shared["wf"][wb][:, 512 * nh : 512 * (nh + 1)],
            start=(wb == 0),
            stop=(wb == NKB - 1),
        )
    fo = spool.tile([128, 512], F32, tag="fo")
    nc.vector.tensor_add(
        fo[:], pf[:], shared["bias_f"][:, 512 * nh : 512 * (nh + 1)]
    )
    nc.sync.dma_start(
        fin["out"][128 * tb : 128 * (tb + 1), 512 * nh : 512 * (nh + 1)], fo[:]
    )


def _build_rep(nc, tc, rep, pk_d, out_d):
    with tc.tile_pool(name=f"shared{rep}", bufs=1) as sp:
        pkS1 = sp.tile([128, LEN_S1], DT, tag="pkS1")
        nc.sync.dma_start(pkS1[:], pk_d[:, 0:LEN_S1])
        pkS2 = sp.tile([128, LEN_S - LEN_S1], DT, tag="pkS2")
        nc.sync.dma_start(pkS2[:], pk_d[:, LEN_S1:LEN_S])

        shared = {
            "wk": [pkS1[:, S_WK + HD * i : S_WK + HD * (i + 1)] for i in range(NKB)],
            "wv": [pkS1[:, S_WV + HD * i : S_WV + HD * (i + 1)] for i in range(NKB)],
            "wq": [pkS1[:, S_WQ + WIDTH * i : S_WQ + WIDTH * (i + 1)] for i in range(NKB)],
            "tri": pkS1[:, S_TRI : S_TRI + 512],
            "bias": pkS1[:, S_BIAS : S_BIAS + WIDTH],
            "wf": [pkS2[:, WIDTH * i : WIDTH * (i + 1)] for i in range(NKB)],
        }
        ident = sp.tile([128, 128], DT, tag="ident")
        make_identity(nc, ident[:])
        bias_f = sp.tile([128, WIDTH], F32, tag="biasf")
        nc.scalar.copy(bias_f[:], shared["bias"])
        shared["bias_f"] = bias_f
        shared["ident"] = ident

        # rolling per-chunk in-flight state (activation slice + qT tiles):
        # a bufs=2 pool rotates two physical buffers, so state(c+1) can be
        # DMA'd and Q-projected while state(c) is still being consumed.
        with (
            tc.tile_pool(name=f"flight{rep}", bufs=2) as flight,
            tc.tile_pool(name=f"loc{rep}", bufs=2) as loc,
        ):
            states = [None] * CHUNKS

            def open_state(c):
                pkX = flight.tile([128, LEN_CK], DT, tag="pkX")
                nc.sync.dma_start(
                    pkX[:], pk_d[:, LEN_S + c * LEN_CK : LEN_S + (c + 1) * LEN_CK]
                )
                states[c] = {
                    "pkX": pkX,
                    "qT": [
                        flight.tile([128, T], DT, tag=f"qT{i}", name=f"qT{i}")
                        for i in range(NKB)
                    ],
                    "xkv": [pkX[:, KV * i : KV * (i + 1)] for i in range(NKB)],
                    "valid": pkX[:, CK_VALID : CK_VALID + NSB],
                }

            open_state(0)
            with tc.tile_pool(name=f"psq{rep}i", bufs=3, space="PSUM") as psq0:
                for mb in range(NKB):
                    _emit_qproj_block(nc, shared, states[0], psq0, mb)

            fin = None
            for c in range(CHUNKS):
                if c + 1 < CHUNKS:
                    open_state(c + 1)
                fin = _build_body(
                    nc, tc, rep, out_d, shared, c, loc, states[c],
                    states[c + 1] if c + 1 < CHUNKS else None, fin,
                )
                states[c] = None
            # epilogue: transpose + final projection of the last chunk
            _emit_attnT(nc, tc, rep, CHUNKS - 1, shared, fin)
            with (
                tc.tile_pool(name=f"psf{rep}e", bufs=4, space="PSUM") as psf_pool,
                tc.tile_pool(name=f"fin{rep}e", bufs=3) as fin_pool,
            ):
                for mb in range(2 * NTB):
                    _emit_final_block(nc, shared, fin, psf_pool, fin_pool, mb)


def _emit_attnT(nc, tc, rep, c, shared, fin):
    """PE-transpose a finished chunk's attn tiles into its attnT tiles."""
    with tc.tile_pool(name=f"psat{rep}c{c}", bufs=2, space="PSUM") as psat_pool:
        for wb in range(NKB):
            pat = psat_pool.tile([128, NTB, 128], DT, tag="pat")
            for tb in range(NTB):
                nc.tensor.transpose(
                    pat[:, tb, :],
                    fin["attn"][tb][:, 128 * wb : 128 * (wb + 1)],
                    shared["ident"][:],
                )
            nc.vector.tensor_copy(fin["attnT"][wb][:], pat[:])


def _build_body(nc, tc, rep, out_d, shared, c, loc, state, next_state, prev_fin):
    row0 = c * T
    wk, wv, wf = shared["wk"], shared["wv"], shared["wf"]
    tri, bias_f, ident = shared["tri"], shared["bias_f"], shared["ident"]
    xkv, valid, qT_t = state["xkv"], state["valid"], state["qT"]

    if True:
        ktd = loc.tile([128, KV], DT, tag="ktd")
        vaug = loc.tile([128, NSB, HD + 1], DT, tag="vaug")
        attn_t = [
            loc.tile([128, WIDTH], DT, tag=f"attn{i}", name=f"attn{i}")
            for i in range(NTB)
        ]
        attnT_t = [
            loc.tile([128, T], DT, tag=f"attnT{i}", name=f"attnT{i}")
            for i in range(NKB)
        ]

        # ---- K/V projections ----
        with (
            tc.tile_pool(name=f"psk{rep}c{c}", bufs=1, space="PSUM") as psk_pool,
            tc.tile_pool(name=f"psv{rep}c{c}", bufs=1, space="PSUM") as psv_pool,
        ):
            pk_ps = psk_pool.tile([128, KV], F32, tag="pk")
            for half in (0, 64):
                for seg0, segw in ((0, 512), (512, 256)):
                    for kb in range(NKB):
                        nc.tensor.matmul(
                            pk_ps[half : half + 64, seg0 : seg0 + segw],
                            lhsT=wk[kb],
                            rhs=xkv[kb][:, seg0 : seg0 + segw],
                            start=(kb == 0),
                            stop=(kb == NKB - 1),
                        )
            nc.vector.tensor_copy(ktd[:], pk_ps[:])

            pv = psv_pool.tile([128, NSB, HD], F32, tag="pv")
            for sb in range(NSB):
                for kb in range(NKB):
                    nc.tensor.matmul(
                        pv[:, sb, :],
                        lhsT=xkv[kb][:, 128 * sb : 128 * (sb + 1)],
                        rhs=wv[kb],
                        start=(kb == 0),
                        stop=(kb == NKB - 1),
                    )
            nc.scalar.copy(vaug[:, :, 0:HD], pv[:])
            nc.vector.tensor_copy(
                vaug[:, :, HD : HD + 1], valid.rearrange("p (s o) -> p s o", o=1)
            )

        if prev_fin is not None:
            _emit_attnT(nc, tc, rep, c - 1, shared, prev_fin)

        # ---- attention, with the next chunk's Q-projection AND the previous
        # chunk's final projection software-pipelined into the head-pair loop
        # (one block of each per pair; PE fills ACT/DVE gaps). PSUM: psl 4 +
        # pso 2 + psqn 1 + psfn 1 = 8/8 banks.
        J2K3 = (0, 2, 1)
        with (
            tc.tile_pool(name=f"psl{rep}c{c}", bufs=2, space="PSUM") as psl_pool,
            tc.tile_pool(name=f"pso{rep}c{c}", bufs=2, space="PSUM") as pso_pool,
            tc.tile_pool(name=f"psqn{rep}c{c}", bufs=1, space="PSUM") as psqn_pool,
            tc.tile_pool(name=f"psfn{rep}c{c}", bufs=1, space="PSUM") as psfn_pool,
            tc.tile_pool(name=f"awork{rep}c{c}", bufs=3) as awork,
        ):
            for mb in range(NKB):  # head pair (2*mb, 2*mb+1)
                if next_state is not None:
                    _emit_qproj_block(nc, shared, next_state, psqn_pool, mb)
                if prev_fin is not None:
                    _emit_final_block(nc, shared, prev_fin, psfn_pool, awork, mb)
                qh = qT_t[mb]
                for tb in range(NTB):
                    pl = psl_pool.tile([128, 2, 4, 128], F32, tag="pl")
                    for half in (0, 1):
                        hb = 64 * half
                        for j in range(3):
                            sb = tb + J2K3[j]
                            nc.tensor.matmul(
                                pl[:, half, j, :],
                                lhsT=ktd[hb : hb + 64, 128 * sb : 128 * (sb + 1)],
                                rhs=qh[hb : hb + 64, 128 * tb : 128 * (tb + 1)],
                                start=True,
                                stop=True,
                            )
                    probsT = awork.tile([128, 2, 3, 128], DT, tag="probsT")
                    nc.scalar.activation(
                        out=probsT[:].rearrange("p h j t -> p h (j t)"),
                        in_=pl[:, :, 0:3, :].rearrange("p h j t -> p h (j t)"),
                        func=mybir.ActivationFunctionType.Exp,
                        scale=0.125,
                    )
                    for half in (0, 1):
                        nc.vector.tensor_mul(
                            probsT[:, half, 0:2, :],
                            probsT[:, half, 0:2, :],
                            tri[:, 0:256].rearrange("p (j t) -> p j t", j=2),
                        )
                    po = pso_pool.tile([128, 2, 128], F32, tag="po")
                    for half in (0, 1):
                        for j in range(3):
                            k3 = J2K3[j]
                            nc.tensor.matmul(
                                po[:, half, 0 : HD + 1],
                                lhsT=probsT[:, half, j, :],
                                rhs=vaug[:, tb + k3, :],
                                start=(j == 0),
                                stop=(j == 2),
                            )
                    recip = awork.tile([128, 2, 1], F32, tag="recip")
                    nc.vector.reciprocal(recip[:], po[:, :, HD : HD + 1])
                    for half in (0, 1):
                        h = 2 * mb + half
                        nc.vector.tensor_scalar_mul(
                            attn_t[tb][:, 64 * h : 64 * (h + 1)],
                            po[:, half, 0:HD],
                            recip[:, half, :],
                        )

        return {
            "attn": attn_t,
            "attnT": attnT_t,
            "out": out_d[row0 : row0 + T, :],
        }


def prep_inputs(x, Wq, Wk, Wv, Wf, bf):
    """Pack everything into one [128, PK_COLS] bf16 array."""
    pk = np.zeros((128, PK_COLS), np.float32)
    for i in range(NKB):
        pk[:, S_WK + HD * i : S_WK + HD * (i + 1)] = Wk.T[128 * i : 128 * (i + 1)]
        pk[:, S_WV + HD * i : S_WV + HD * (i + 1)] = Wv.T[128 * i : 128 * (i + 1)]
        pk[:, S_WQ + WIDTH * i : S_WQ + WIDTH * (i + 1)] = Wq.T[128 * i : 128 * (i + 1)]
        pk[:, S_WF + WIDTH * i : S_WF + WIDTH * (i + 1)] = Wf.T[128 * i : 128 * (i + 1)]

    s = np.arange(128)[:, None]
    t = np.arange(128)[None, :]
    tri_lo = (s >= t).astype(np.float32)  # k3=0 block: keep s >= t
    tri_hi = (s <= t).astype(np.float32)  # k3=2 block: keep s <= t
    pk[:, S_TRI + 0 : S_TRI + 128] = tri_lo
    pk[:, S_TRI + 128 : S_TRI + 256] = tri_hi
    pk[:, S_TRI + 256 : S_TRI + 384] = tri_lo
    pk[:, S_TRI + 384 : S_TRI + 512] = tri_hi
    pk[:, S_BIAS : S_BIAS + WIDTH] = np.broadcast_to(bf.astype(np.float32), (128, WIDTH))

    for c in range(CHUNKS):
        bi, ch = divmod(c, 4)
        qs = T * ch
        ks = qs - WIN
        base = LEN_S + c * LEN_CK
        xkvT = np.zeros((WIDTH, KV), np.float32)
        lo = max(ks, 0)
        xkvT[:, lo - ks :] = x[bi, lo : qs + T, :].T
        for i in range(NKB):
            pk[:, base + CK_XKV + KV * i : base + CK_XKV + KV * (i + 1)] = xkvT[
                128 * i : 128 * (i + 1)
            ]
        kv_pos = ks + (np.arange(NSB)[None, :] * 128 + np.arange(128)[:, None])
        pk[:, base + CK_VALID : base + CK_VALID + NSB] = (kv_pos >= 0).astype(
            np.float32
        )
    return {"pk": pk.astype(NPDT)}


_RUN_KW = {}  # test.py can inject trace=True etc.
_LAST_RESULT = [None]


def kernel(x, segment_pos, Wq, Wk, Wv, Wf, bf):
    x = np.asarray(x, np.float32)
    Wq = np.asarray(Wq, np.float32)
    Wk = np.asarray(Wk, np.float32)
    Wv = np.asarray(Wv, np.float32)
    Wf = np.asarray(Wf, np.float32)
    bf = np.asarray(bf, np.float32)

    nc = build_kernel()
    nc.finalize()
    in_maps = [prep_inputs(x, Wq, Wk, Wv, Wf, bf)]
    res = run_bass_kernel_spmd(nc, in_maps, core_ids=[0], **_RUN_KW)
    _LAST_RESULT[0] = res

    b, t = x.shape[0], x.shape[1]
    return np.ascontiguousarray(
        res.results[0]["out"].reshape(b, t, WIDTH).astype(np.float32)
    )
